# revision 13
# baseline (speedup 1.0000x reference)
"""Trainium2 Bass kernel for a 2-layer GAT (PyG GATConv x2, eval mode).

Strategy (8 NeuronCores, SPMD single program):
  - Host: add self-loops, sort destinations by in-degree, pack into groups of
    128 similar-degree dsts in one CANONICAL core-major order shared by both
    layers (one gather-index set serves layer 1 and layer 2). Per-group edge
    slots [128 dsts x degree] split LO/HI by table row (int16 gather indices;
    rows >= 32768 gather from a shifted table base).
  - Phase A (sharded): each core computes its own 1/8 of the layer-1 node
    table: T1row = [h fp8e4m3 x128 | s_src bf16 x4 | pad] (256B rows, halves
    gather descriptor size vs bf16), s_dst kept in SBUF. AllGather -> full T1.
  - Phase B (edge pass 1): per dst-group bulk dma_gather of fp8 source rows,
    scores (leaky-relu/exp), exp-weighted messages summed over neighbors via
    identity-matmul PSUM accumulation, then batched softmax-normalize + bias
    + ELU. Pad rows have s_src=-1000 so exp underflows to zero - no masks.
  - Phase C: layer-2 GEMM on own rows, AllGather the layer-2 table (f32,
    256B rows).
  - Phase D: edge pass 2 (heads=1) reusing the same gather indices,
    normalize + bias, write output chunk.
  - Host: gather per-core chunks, undo the permutation.
"""

import numpy as np
import ml_dtypes

P = 128
NCORES = 8
SPLIT = 32768          # dma_gather int16 index range per table base

N_NODES = 50000
F_IN = 256
HID = 32
HEADS = 4
CLS = 40
NEG_SLOPE = 0.2
PAD_SSRC = -1000.0     # pad-row source score: exp under flows to 0 exactly
EPS = 1e-16
TW1 = 256              # layer-1 table row BYTES (int8): h fp8 x128, s_src bf16 x4
TW2 = 64               # layer-2 table row f32 elems (256B), cols: h(40) s(1) pad
MAXSLOT = 8            # dma_gather caps at 1024 indices per call


class Plan:
    pass


def _pack16(arr):
    """[128, SD] int -> dma_gather idx layout [128, 8*SD]: index i=(c*128+p)
    at [p%16, c*8 + p//16], and the 16-partition pattern replicated 8x
    across partitions (one copy per Q7 core)."""
    p128, sd = arr.shape
    assert p128 == 128
    base = np.ascontiguousarray(
        arr.reshape(8, 16, sd).transpose(1, 2, 0).reshape(16, sd * 8)
    ).astype(np.int16)
    return np.tile(base, (8, 1))


def make_plan(edge_index, n_nodes, ncores=NCORES):
    import math

    src_all = np.concatenate([edge_index[0], np.arange(n_nodes, dtype=np.int64)])
    dst_all = np.concatenate([edge_index[1], np.arange(n_nodes, dtype=np.int64)])

    deg = np.bincount(dst_all, minlength=n_nodes)
    ngrp = math.ceil(n_nodes / P)
    ngrp = math.ceil(ngrp / ncores) * ncores
    nslot = ngrp * P
    ndum = nslot - n_nodes
    ngc = ngrp // ncores
    nloc = ngc * P
    n_t = nslot + 2                 # rows: [pad_lo, nodes..., pad_hi]
    pad_lo_row = 0
    pad_hi_row = nslot + 1
    # hi gather window starts here so [BHI, 32768) is reachable from BOTH
    # windows - that slack lets per-dst lo/hi counts be balanced to kill
    # the rectangle padding.
    BHI = max(n_t - SPLIT, 0)

    es = np.argsort(dst_all, kind="stable")
    srcs_sorted = src_all[es]
    dsts_sorted = dst_all[es]
    starts = np.concatenate([[0], np.cumsum(deg)])

    # pass 0: degree sort + core-major interleave -> canonical positions.
    # Count per-dst window-mandatory sources against those rows, then
    # re-sort nodes WITHIN each canonical window-region (membership, hence
    # counts, invariant) so every group is homogeneous in (deg, Amin-Bmin).
    order0 = np.argsort(deg, kind="stable")
    slot_node0 = np.concatenate([np.full(ndum, -1, np.int64), order0])
    groups0 = slot_node0.reshape(ngrp, P)
    can0 = np.empty((ncores, ngc, P), np.int64)
    for k in range(ncores):
        can0[k] = groups0[k::ncores]
    can_nodes0 = can0.reshape(-1)
    pos0 = np.full(n_nodes, -1, np.int64)
    m0 = can_nodes0 >= 0
    pos0[can_nodes0[m0]] = np.nonzero(m0)[0]
    rows_e = pos0[srcs_sorted] + 1
    Amin = np.zeros(n_nodes, np.int64)
    Bmin = np.zeros(n_nodes, np.int64)
    np.add.at(Amin, dsts_sorted, rows_e < BHI)
    np.add.at(Bmin, dsts_sorted, rows_e >= SPLIT)
    key = Amin - Bmin

    canpos = np.arange(nslot)
    can_row = canpos + 1
    reg_of = np.where(can_row < BHI, 0, np.where(can_row < SPLIT, 1, 2))
    k_of = canpos // nloc
    gi_of = (canpos % nloc) // P
    p_of = canpos % P
    can_nodes = np.full(nslot, -1, np.int64)
    for r in range(3):
        sel = np.nonzero(reg_of == r)[0]
        so = sel[np.lexsort((p_of[sel], k_of[sel], gi_of[sel]))]
        nodes = can_nodes0[sel]
        nv = nodes[nodes >= 0]
        nv = nv[np.lexsort((key[nv], deg[nv]))]
        fill = np.concatenate([np.full(len(nodes) - len(nv), -1, np.int64), nv])
        can_nodes[so] = fill
    can_slot = can_nodes.reshape(ncores, ngc, P)
    pos_can = np.full(n_nodes, -1, np.int64)
    m2 = can_nodes >= 0
    pos_can[can_nodes[m2]] = np.nonzero(m2)[0]

    # counts are region-stable; recompute from final rows (safety)
    rows_f = pos_can[srcs_sorted] + 1
    Amin = np.zeros(n_nodes, np.int64)
    Bmin = np.zeros(n_nodes, np.int64)
    np.add.at(Amin, dsts_sorted, rows_f < BHI)
    np.add.at(Bmin, dsts_sorted, rows_f >= SPLIT)

    # per-group balanced A-target sweep (jointly over the 8 cores' dsts)
    DA = np.zeros(ngc, np.int64)
    DB = np.zeros(ngc, np.int64)
    Tg = np.zeros(ngc, np.int64)
    for gi in range(ngc):
        vv = can_slot[:, gi, :].reshape(-1)
        vv0 = np.maximum(vv, 0)
        am = np.where(vv >= 0, Amin[vv0], 0)
        bm = np.where(vv >= 0, Bmin[vv0], 0)
        dg = np.where(vv >= 0, deg[vv0], 0)
        amax = dg - bm
        best = None
        bestc = None
        for T in range(int(am.max()), int(amax.max()) + 1):
            A = np.clip(T, am, amax)
            da = max(T, int(am.max()))
            db = int((dg - A).max())
            c = (da + db) + 6 * (-(-da // MAXSLOT) + -(-db // MAXSLOT))
            if bestc is None or c < bestc:
                best, bestc = (da, db, T), c
        DA[gi], DB[gi], Tg[gi] = best
    DA = np.maximum(DA, 1)
    offA = np.concatenate([[0], np.cumsum(DA)])
    offB = np.concatenate([[0], np.cumsum(DB)])
    SDA, SDB = int(offA[-1]), int(offB[-1])

    padA = pad_lo_row
    padB = pad_hi_row - BHI
    assert padB < SPLIT
    idxlo = np.full((ncores, P, SDA), padA, np.int64)
    idxhi = np.full((ncores, P, SDB), padB, np.int64)
    for k in range(ncores):
        for gi in range(ngc):
            da, db = int(DA[gi]), int(DB[gi])
            T = int(Tg[gi])
            vv = can_slot[k, gi]
            for p in range(P):
                v = vv[p]
                if v < 0:
                    continue
                d = int(deg[v])
                rr = pos_can[srcs_sorted[starts[v]:starts[v] + d]] + 1
                manA = rr[rr < BHI]
                manB = rr[rr >= SPLIT]
                fx = rr[(rr >= BHI) & (rr < SPLIT)]
                a_cnt = min(max(T, len(manA)), d - len(manB))
                take = a_cnt - len(manA)
                arows = np.concatenate([manA, fx[:take]])
                brows = np.concatenate([fx[take:], manB])
                assert len(arows) <= da and len(brows) <= db, (k, gi, p)
                idxlo[k, p, offA[gi]:offA[gi] + len(arows)] = arows
                idxhi[k, p, offB[gi]:offB[gi] + len(brows)] = brows - BHI
    lo16 = np.stack([_pack16(idxlo[k]) for k in range(ncores)])
    hi16 = (np.stack([_pack16(idxhi[k]) for k in range(ncores)])
            if SDB else np.zeros((ncores, 128, 8), np.int16))
    L = dict(DA=DA, DB=DB, offA=offA, offB=offB, SDA=SDA, SDB=SDB,
             lo16=lo16, hi16=hi16)

    pl = Plan()
    pl.ncores = ncores
    pl.n_nodes = n_nodes
    pl.ngrp, pl.nslot, pl.ngc, pl.nloc, pl.n_t = ngrp, nslot, ngc, nloc, n_t
    pl.BHI = BHI
    pl.L = L
    pl.can_slot = can_slot
    return pl


def make_inputs(pl, x, W1, att_src1, att_dst1, b1, W2, att_src2, att_dst2, b2):
    f_in = x.shape[1]
    fh = W1.shape[1]
    heads = att_src1.shape[0]
    hid = fh // heads
    cls = W2.shape[1]
    cw1g = fh + 2 * heads
    cw2g = cls + 2

    asrc = np.zeros((fh, heads), np.float32)
    adst = np.zeros((fh, heads), np.float32)
    for h in range(heads):
        asrc[h * hid:(h + 1) * hid, h] = att_src1[h]
        adst[h * hid:(h + 1) * hid, h] = att_dst1[h]
    W1a = np.concatenate([W1, W1 @ asrc, W1 @ adst], axis=1)
    kt = f_in // P
    W1a = W1a.reshape(kt, P, cw1g).astype(ml_dtypes.bfloat16)

    W2a = np.concatenate(
        [W2, (W2 @ att_src2[0])[:, None], (W2 @ att_dst2[0])[:, None]], axis=1
    ).astype(np.float32)

    # layer-1 pad row: h fp8 zeros, s_src bf16 -1000 at bytes [128,136)
    pad1 = np.zeros(TW1, np.int8)
    pad1[128:136] = np.full(4, PAD_SSRC, ml_dtypes.bfloat16).view(np.int8)
    pad1 = pad1[None, :]
    pad2 = np.zeros((1, TW2), np.float32)
    pad2[0, cls] = PAD_SSRC

    b1t = np.tile(b1[None, :], (P, 1)).astype(np.float32)
    b2t = np.tile(b2[None, :], (P, 1)).astype(np.float32)
    idbf = np.eye(P).astype(ml_dtypes.bfloat16)
    idf = np.eye(P, dtype=np.float32)

    in_maps = []
    for k in range(pl.ncores):
        o = pl.can_slot[k].reshape(-1)
        xtab = np.zeros((pl.nloc, f_in), np.float32)
        mm = o >= 0
        xtab[mm] = x[o[mm]]
        xT = np.ascontiguousarray(xtab.T).astype(ml_dtypes.bfloat16)
        in_maps.append({
            "xT": xT, "W1a": W1a, "W2a": W2a,
            "ilo": pl.L["lo16"][k], "ihi": pl.L["hi16"][k],
            "pad1": pad1, "pad2": pad2,
            "b1t": b1t, "b2t": b2t, "idbf": idbf, "idf": idf,
        })
    return in_maps


# ------------------------------------------------------------- bass program

def build_bass(pl, f_in=F_IN, heads=HEADS, hid=HID, cls=CLS, dbg=False,
               stop_after=None, reps=1, maxslot=MAXSLOT):
    import concourse.bass as bass
    import concourse.bacc as bacc
    import concourse.tile as tile
    from concourse import mybir

    fh = heads * hid
    cw1g = fh + 2 * heads
    cw2 = cls + 1
    cw2g = cls + 2
    kt = f_in // P
    ngc, nslot, nloc, n_t = pl.ngc, pl.nslot, pl.nloc, pl.n_t
    L = pl.L
    core_ids = list(range(pl.ncores))

    f32, bf16 = mybir.dt.float32, mybir.dt.bfloat16
    i16, i8 = mybir.dt.int16, mybir.dt.int8
    fp8 = mybir.dt.float8e4
    AF = mybir.ActivationFunctionType
    OP = mybir.AluOpType

    nc = bacc.Bacc("TRN2", target_bir_lowering=False, debug=False,
                   num_swdge_queues=4)

    xT = nc.declare_dram_parameter("xT", [f_in, nloc], bf16, isOutput=False)
    W1a = nc.declare_dram_parameter("W1a", [kt, P, cw1g], bf16, isOutput=False)
    W2a = nc.declare_dram_parameter("W2a", [fh, cw2g], f32, isOutput=False)
    ilo = nc.declare_dram_parameter("ilo", list(L["lo16"].shape[1:]), i16, isOutput=False)
    ihi = nc.declare_dram_parameter("ihi", list(L["hi16"].shape[1:]), i16, isOutput=False)
    pad1 = nc.declare_dram_parameter("pad1", [1, TW1], i8, isOutput=False)
    pad2 = nc.declare_dram_parameter("pad2", [1, TW2], f32, isOutput=False)
    b1t = nc.declare_dram_parameter("b1t", [P, fh], f32, isOutput=False)
    b2t = nc.declare_dram_parameter("b2t", [P, cls], f32, isOutput=False)
    idbf = nc.declare_dram_parameter("idbf", [P, P], bf16, isOutput=False)
    idf = nc.declare_dram_parameter("idf", [P, P], f32, isOutput=False)

    out2d = nc.declare_dram_parameter("out2d", [nloc, cls], f32, isOutput=True)
    if dbg:
        t1o = nc.declare_dram_parameter("t1o", [n_t, TW1], i8, isOutput=True)
        h1o = nc.declare_dram_parameter("h1o", [P, ngc * fh], f32, isOutput=True)
        dso = nc.declare_dram_parameter("dso", [P, ngc * heads], f32, isOutput=True)
        t2o = nc.declare_dram_parameter("t2o", [n_t, TW2], f32, isOutput=True)

    T1chunk = nc.dram_tensor("T1chunk", [nloc, TW1], i8)
    T1 = nc.dram_tensor("T1", [n_t, TW1], i8, addr_space="Shared")
    T2chunk = nc.dram_tensor("T2chunk", [nloc, TW2], f32)
    T2 = nc.dram_tensor("T2", [n_t, TW2], f32, addr_space="Shared")

    def ap_of(t, offset, dims):
        a = t[:]
        part = list(a.ap[0])
        return bass.AP(a.tensor, a.offset + offset, [part] + [list(d) for d in dims])

    def mkap(t, offset, dims):
        a = t[:]
        return bass.AP(a.tensor, a.offset + offset, [list(d) for d in dims])

    qctr = [0]

    def gather(out_tile, slot_off, nslots, table, tw, idx_tile, idx_off, base_row):
        """dma_gather nslots*128 rows of width tw into out_tile at slot_off."""
        in_ap = mkap(table, base_row * tw, [[tw, n_t - base_row], [1, tw]])
        done = 0
        while done < nslots:
            cn = min(maxslot, nslots - done)
            out_ap = ap_of(out_tile, (slot_off + done) * tw, [[tw, cn], [1, tw]])
            idx_ap = idx_tile[:, (idx_off + done) * 8:(idx_off + done + cn) * 8]
            n = cn * P
            nc.gpsimd.dma_gather(
                out_ap=out_ap, in_ap=in_ap, idxs_ap=idx_ap,
                num_idxs=n, num_idxs_reg=n, elem_size=tw,
                queue_num=qctr[0] % 4,
            )
            qctr[0] += 1
            done += cn

    ablk = 7
    while ngc % ablk:
        ablk -= 1
    nblk = ngc // ablk

    with tile.TileContext(nc) as tc:
        with tc.tile_pool(name="stage", bufs=1) as stage:
            idbf_t = stage.tile([P, P], bf16)
            nc.sync.dma_start(out=idbf_t[:], in_=idbf[:, :])
            idf_t = stage.tile([P, P], f32)
            nc.sync.dma_start(out=idf_t[:], in_=idf[:, :])
            b1t_t = stage.tile([P, fh], f32)
            nc.sync.dma_start(out=b1t_t[:], in_=b1t[:, :])
            b2t_t = stage.tile([P, cls], f32)
            nc.sync.dma_start(out=b2t_t[:], in_=b2t[:, :])
            ilo_t = stage.tile([P, L["lo16"].shape[2]], i16)
            nc.sync.dma_start(out=ilo_t[:], in_=ilo[:, :])
            ihi_t = stage.tile([P, L["hi16"].shape[2]], i16)
            nc.sync.dma_start(out=ihi_t[:], in_=ihi[:, :])

            numstage = stage.tile([P, ngc * fh], f32)
            tmpstage = stage.tile([P, ngc * fh], f32)
            dstage = stage.tile([P, ngc * heads], f32)
            sdst_sb = stage.tile([P, ngc * heads], f32)
            s2d = stage.tile([P, ngc], f32)
            d2stage = stage.tile([P, ngc], f32)
            o2stage = stage.tile([P, ngc * cls], f32)

            for _rep in range(reps):
                # ------------------------------------------------ phase A: T1 GEMM
                with (
                    tc.tile_pool(name="pa", bufs=1) as pa,
                    tc.tile_pool(name="psA", bufs=4, space="PSUM") as psA,
                ):
                    w1_sb = pa.tile([P, kt * cw1g], bf16)
                    nc.sync.dma_start(
                        out=w1_sb[:], in_=W1a[:, :, :].transpose([1, 0, 2])
                    )
                    xa = []
                    for kk in range(kt):
                        t = pa.tile([P, nloc], bf16, tag=f"xa{kk}")
                        nc.sync.dma_start(
                            out=t[:], in_=xT[kk * P:(kk + 1) * P, :]
                        )
                        xa.append(t)
                    nc.sync.dma_start(out=T1[0:1, :], in_=pad1[:, :])
                    nc.sync.dma_start(out=T1[nslot + 1:nslot + 2, :], in_=pad1[:, :])
                    nc.sync.dma_start(out=T2[0:1, :], in_=pad2[:, :])
                    nc.sync.dma_start(out=T2[nslot + 1:nslot + 2, :], in_=pad2[:, :])

                    for blk in range(nblk):
                        g0 = blk * ablk
                        stb = pa.tile([P, ablk * TW1], i8, tag="ast", bufs=2)
                        for m0 in range(0, ablk, 3):
                            nm = min(3, ablk - m0)
                            pt = psA.tile([P, nm * cw1g], f32)
                            for i in range(nm):
                                for kk in range(kt):
                                    nc.tensor.matmul(
                                        out=pt[:, i * cw1g:(i + 1) * cw1g],
                                        lhsT=xa[kk][:, (g0 + m0 + i) * P:(g0 + m0 + i + 1) * P],
                                        rhs=w1_sb[:, kk * cw1g:(kk + 1) * cw1g],
                                        start=(kk == 0),
                                        stop=(kk == kt - 1),
                                    )
                            nc.scalar.activation(
                                out=ap_of(stb, m0 * TW1,
                                          [[TW1, nm], [1, fh]]).bitcast(fp8),
                                in_=ap_of(pt, 0, [[cw1g, nm], [1, fh]]),
                                func=AF.Copy,
                            )
                            nc.scalar.activation(
                                out=ap_of(stb, m0 * TW1 + fh,
                                          [[TW1, nm], [1, 2 * heads]]).bitcast(bf16),
                                in_=ap_of(pt, fh, [[cw1g, nm], [1, heads]]),
                                func=AF.Copy,
                            )
                            nc.scalar.activation(
                                out=sdst_sb[:, (g0 + m0) * heads:(g0 + m0 + nm) * heads],
                                in_=ap_of(pt, fh + heads, [[cw1g, nm], [1, heads]]),
                                func=AF.Copy,
                            )
                        nc.sync.dma_start(
                            out=mkap(T1chunk, g0 * P * TW1,
                                     [[TW1, P], [P * TW1, ablk], [1, TW1]]),
                            in_=ap_of(stb, 0, [[TW1, ablk], [1, TW1]]),
                        )
                    nc.gpsimd.collective_compute(
                        "AllGather",
                        OP.bypass,
                        replica_groups=[core_ids],
                        ins=[T1chunk[:, :]],
                        outs=[T1[1:1 + nslot, :]],
                    )

                # ---------------------------------------------- phase B: edge pass 1
                if stop_after == "A":
                    continue
                with (
                    tc.tile_pool(name="pb", bufs=4) as pb,
                    tc.tile_pool(name="pc_w", bufs=1) as pcw,
                    tc.tile_pool(name="psB", bufs=2, space="PSUM") as psB,
                    tc.tile_pool(name="psT", bufs=2, space="PSUM") as psT,
                    tc.tile_pool(name="psC", bufs=4, space="PSUM") as psC,
                ):
                    w2_sb = pcw.tile([P, cw2g], f32)
                    nc.sync.dma_start(out=w2_sb[:], in_=W2a[:, :])
                    for gi in range(ngc):
                        da, db = int(L["DA"][gi]), int(L["DB"][gi])
                        d = da + db
                        m_t = pb.tile([P, d * TW1], i8, tag="m")
                        gather(m_t, 0, da, T1, TW1, ilo_t, int(L["offA"][gi]), 0)
                        if db:
                            gather(m_t, da, db, T1, TW1, ihi_t,
                                   int(L["offB"][gi]), pl.BHI)
                        ssum = pb.tile([P, d * heads], f32, tag="ss")
                        nc.vector.tensor_tensor(
                            out=ssum[:],
                            in0=ap_of(m_t, fh, [[TW1, d], [1, 2 * heads]]).bitcast(bf16),
                            in1=ap_of(sdst_sb, gi * heads, [[0, d], [1, heads]]),
                            op=OP.add,
                        )
                        tmp = pb.tile([P, d * heads], f32, tag="tm")
                        nc.vector.tensor_scalar_mul(
                            out=tmp[:], in0=ssum[:], scalar1=NEG_SLOPE
                        )
                        nc.vector.tensor_tensor(
                            out=ssum[:], in0=ssum[:], in1=tmp[:], op=OP.max
                        )
                        ex = pb.tile([P, d * heads], bf16, tag="ex")
                        nc.scalar.activation(out=ex[:], in_=ssum[:], func=AF.Exp)
                        nc.vector.tensor_reduce(
                            out=dstage[:, gi * heads:(gi + 1) * heads],
                            in_=ap_of(ex, 0, [[1, heads], [heads, d]]),
                            axis=mybir.AxisListType.X,
                            op=OP.add,
                        )
                        mw = pb.tile([P, d * fh], bf16, tag="mw")
                        nc.vector.tensor_tensor(
                            out=mw[:],
                            in0=ap_of(m_t, 0,
                                      [[TW1, d], [hid, heads], [1, hid]]).bitcast(fp8),
                            in1=ap_of(ex, 0, [[heads, d], [1, heads], [0, hid]]),
                            op=OP.mult,
                        )
                        pn = psB.tile([P, fh], f32)
                        for j in range(d):
                            nc.tensor.matmul(
                                out=pn[:],
                                lhsT=idbf_t[:],
                                rhs=mw[:, j * fh:(j + 1) * fh],
                                start=(j == 0),
                                stop=(j == d - 1),
                            )
                        nc.scalar.activation(
                            out=numstage[:, gi * fh:(gi + 1) * fh],
                            in_=pn[:], func=AF.Copy,
                        )

                        q = ngc // 4
                        cuts = (q - 1, 2 * q - 1, 3 * q - 1, ngc - 1)
                        if gi not in cuts:
                            continue
                        # quarter-table tail: out1 = elu(num/den + b1), then
                        # transpose + layer-2 GEMM + chunk store for the slice
                        # just finished (overlaps later groups' gathers)
                        hi_g = gi + 1
                        lo_g = ([0] + [c + 1 for c in cuts])[cuts.index(gi)]
                        cnt = hi_g - lo_g
                        dsl = dstage[:, lo_g * heads:hi_g * heads]
                        nsl = numstage[:, lo_g * fh:hi_g * fh]
                        tsl = tmpstage[:, lo_g * fh:hi_g * fh]
                        nc.vector.tensor_scalar_add(out=dsl, in0=dsl, scalar1=EPS)
                        nc.vector.reciprocal(out=dsl, in_=dsl)
                        nc.vector.tensor_tensor(
                            out=nsl,
                            in0=ap_of(numstage, lo_g * fh,
                                      [[fh, cnt], [hid, heads], [1, hid]]),
                            in1=ap_of(dstage, lo_g * heads,
                                      [[heads, cnt], [1, heads], [0, hid]]),
                            op=OP.mult,
                        )
                        nc.vector.tensor_tensor(
                            out=nsl,
                            in0=nsl,
                            in1=ap_of(b1t_t, 0, [[0, cnt], [1, fh]]),
                            op=OP.add,
                        )
                        nc.vector.tensor_scalar_min(out=tsl, in0=nsl, scalar1=0.0)
                        nc.scalar.activation(out=tsl, in_=tsl, func=AF.Exp)
                        nc.scalar.activation(out=tsl, in_=tsl, func=AF.Copy,
                                             bias=-1.0)
                        nc.vector.tensor_tensor(out=nsl, in0=nsl, in1=tsl,
                                                op=OP.max)
                        for gj in range(lo_g, hi_g):
                            ptr = psT.tile([P, fh], f32)
                            nc.tensor.transpose(
                                out=ptr[:],
                                in_=numstage[:, gj * fh:(gj + 1) * fh],
                                identity=idf_t[:],
                            )
                            nc.scalar.activation(
                                out=numstage[:, gj * fh:(gj + 1) * fh],
                                in_=ptr[:], func=AF.Copy,
                            )
                            pc = psC.tile([P, cw2g], f32)
                            nc.tensor.matmul(
                                out=pc[:],
                                lhsT=numstage[:, gj * fh:(gj + 1) * fh],
                                rhs=w2_sb[:],
                                start=True,
                                stop=True,
                            )
                            stc = pcw.tile([P, cw2g], f32, tag="cst", bufs=3)
                            nc.scalar.activation(out=stc[:], in_=pc[:], func=AF.Copy)
                            nc.sync.dma_start(
                                out=mkap(T2chunk, gj * P * TW2,
                                         [[TW2, P], [1, cw2]]),
                                in_=stc[:, 0:cw2],
                            )
                            nc.scalar.activation(
                                out=s2d[:, gj:gj + 1], in_=stc[:, cw2:cw2 + 1],
                                func=AF.Copy,
                            )
                    nc.gpsimd.collective_compute(
                        "AllGather",
                        OP.bypass,
                        replica_groups=[core_ids],
                        ins=[T2chunk[:, :]],
                        outs=[T2[1:1 + nslot, :]],
                    )

                # ---------------------------------------------- phase D: edge pass 2
                if stop_after in ("C",):
                    continue
                with (
                    tc.tile_pool(name="pd", bufs=4) as pd,
                    tc.tile_pool(name="psD", bufs=2, space="PSUM") as psD,
                ):
                    for gi in range(ngc):
                        da, db = int(L["DA"][gi]), int(L["DB"][gi])
                        d = da + db
                        m2 = pd.tile([P, d * TW2], f32, tag="m2")
                        gather(m2, 0, da, T2, TW2, ilo_t, int(L["offA"][gi]), 0)
                        if db:
                            gather(m2, da, db, T2, TW2, ihi_t,
                                   int(L["offB"][gi]), pl.BHI)
                        ssum = pd.tile([P, d], f32, tag="ss2")
                        nc.vector.tensor_tensor(
                            out=ssum[:],
                            in0=ap_of(m2, cls, [[TW2, d]]),
                            in1=ap_of(s2d, gi, [[0, d]]),
                            op=OP.add,
                        )
                        tmp = pd.tile([P, d], f32, tag="tm2")
                        nc.vector.tensor_scalar_mul(
                            out=tmp[:], in0=ssum[:], scalar1=NEG_SLOPE
                        )
                        nc.vector.tensor_tensor(
                            out=ssum[:], in0=ssum[:], in1=tmp[:], op=OP.max
                        )
                        ex = pd.tile([P, d], f32, tag="ex2")
                        nc.scalar.activation(out=ex[:], in_=ssum[:], func=AF.Exp)
                        nc.vector.tensor_reduce(
                            out=d2stage[:, gi:gi + 1],
                            in_=ex[:],
                            axis=mybir.AxisListType.X,
                            op=OP.add,
                        )
                        mw = pd.tile([P, d * cls], bf16, tag="mw2")
                        nc.vector.tensor_tensor(
                            out=mw[:],
                            in0=ap_of(m2, 0, [[TW2, d], [1, cls]]),
                            in1=ap_of(ex, 0, [[1, d], [0, cls]]),
                            op=OP.mult,
                        )
                        pn = psD.tile([P, cls], f32)
                        for j in range(d):
                            nc.tensor.matmul(
                                out=pn[:],
                                lhsT=idbf_t[:],
                                rhs=mw[:, j * cls:(j + 1) * cls],
                                start=(j == 0),
                                stop=(j == d - 1),
                            )
                        nc.scalar.activation(
                            out=o2stage[:, gi * cls:(gi + 1) * cls],
                            in_=pn[:], func=AF.Copy,
                        )

                        if gi not in (ngc // 2 - 1, ngc - 1):
                            continue
                        hi_g = gi + 1
                        lo_g = 0 if hi_g <= ngc // 2 else ngc // 2
                        cnt = hi_g - lo_g
                        d2s = d2stage[:, lo_g:hi_g]
                        o2s = o2stage[:, lo_g * cls:hi_g * cls]
                        nc.vector.tensor_scalar_add(out=d2s, in0=d2s, scalar1=EPS)
                        nc.vector.reciprocal(out=d2s, in_=d2s)
                        nc.vector.tensor_tensor(
                            out=o2s,
                            in0=ap_of(o2stage, lo_g * cls, [[cls, cnt], [1, cls]]),
                            in1=ap_of(d2stage, lo_g, [[1, cnt], [0, cls]]),
                            op=OP.mult,
                        )
                        nc.vector.tensor_tensor(
                            out=o2s,
                            in0=o2s,
                            in1=ap_of(b2t_t, 0, [[0, cnt], [1, cls]]),
                            op=OP.add,
                        )
                        nc.sync.dma_start(
                            out=mkap(out2d, lo_g * P * cls,
                                     [[cls, P], [P * cls, cnt], [1, cls]]),
                            in_=ap_of(o2stage, lo_g * cls,
                                      [[cls, cnt], [1, cls]]),
                        )

    nc.compile()
    return nc


def assemble_output(pl, results, n_nodes, cls=CLS):
    out = np.zeros((n_nodes, cls), np.float32)
    for k in range(pl.ncores):
        chunk = results[k]["out2d"]
        nodes = pl.can_slot[k].reshape(-1)
        m = nodes >= 0
        out[nodes[m]] = chunk[m]
    return out


# ----------------------------------------------------------------- entry

def kernel(edge_index, x, W1, att_src1, att_dst1, b1, W2, att_src2, att_dst2, b2):
    x = np.asarray(x, np.float32)
    edge_index = np.asarray(edge_index)
    n_nodes = x.shape[0]

    pl = make_plan(edge_index, n_nodes)
    in_maps = make_inputs(pl, x, np.asarray(W1, np.float32),
                          np.asarray(att_src1, np.float32),
                          np.asarray(att_dst1, np.float32),
                          np.asarray(b1, np.float32),
                          np.asarray(W2, np.float32),
                          np.asarray(att_src2, np.float32),
                          np.asarray(att_dst2, np.float32),
                          np.asarray(b2, np.float32))
    nc = build_bass(pl, f_in=x.shape[1], heads=np.asarray(att_src1).shape[0],
                    hid=np.asarray(att_src1).shape[1],
                    cls=np.asarray(W2).shape[1])

    from concourse.bass_utils import run_bass_kernel_spmd
    res = run_bass_kernel_spmd(nc, in_maps, list(range(NCORES)))
    return assemble_output(pl, res.results, n_nodes,
                           cls=np.asarray(W2).shape[1])
